# revision 46
# baseline (speedup 1.0000x reference)
"""Causal self-attention (B=4,T=2048,C=2048,H=16,D=128) on 8 TRN2 NeuronCores.

v8: tensor-parallel over heads (2 heads/core). The PE is GPIO-power-
throttled to ~1.95GHz when all 8 cores stream matmuls, so the design
minimizes streamed PE columns and keeps the PE gap-free at that rate.

Phase 1 computes QKV + RoPE for ALL batches; V is computed directly in
[t, D] layout (x-tile stationary, Wv^T moving) so no PE transposes are
needed. Weight loads ride the scalar HWDGE queue in parallel with x loads
on sync; RoPE's partition-swap DMAs ride gpsimd so they are never stuck
behind ring-blocked x prefetches. Warm-up matmuls on a constant tile bring
the HAM clock-gate to 8/8 during the initial DMA wait.

Phase 2 runs attention in 4 query quarters (jj); after each quarter an
AllToAll redistributes y^T so each core holds all 2048 channels for 256
(b,t) rows of that quarter. op(0) + op(1)-first-half projection groups
interleave into the jj=3 attention stream (its exp-bound stretches need
the PE-heavy/ACT-light filler; none land after the last pair so the A2A3
trigger is not pushed back). op(1)-second-half + op(2) (50us of PE work)
run after the A2A3 trigger, covering even a worst-case-slow A2A (13-43us
observed); only op(3) is exposed at the tail. Diagonal score tiles are
trimmed to their causally-valid column range; the diagonal exp is one
ACTIVATE per group (ACT costs (N+352)/1.2ns, so fewer+wider wins);
masking is one [128,128] constant on the vector engine. bf16 matmuls,
fp32 PSUM, softmax without max-subtraction, bf16 output stores.
"""
import os
import sys

sys.path.insert(0, "/opt/trn_rl_repo")

import numpy as np
import ml_dtypes

B, T, C, H, D = 4, 2048, 2048, 16, 128
NCORES = 8
HPC = H // NCORES          # 2 heads per core
BT = B * T                 # 8192
BTPC = BT // NCORES        # 1024 (b,t) rows per core
KT = C // 128              # 16 contraction tiles
SCALE = 1.0 / float(np.sqrt(D))
BF16 = ml_dtypes.bfloat16

LAST_EXEC_NS = None
_CACHE = {}


def _build_nc():
    from contextlib import ExitStack
    from concourse import bacc, tile, mybir
    from concourse.masks import make_identity

    bf = mybir.dt.bfloat16
    f32 = mybir.dt.float32
    mult = mybir.AluOpType.mult
    add = mybir.AluOpType.add
    Exp = mybir.ActivationFunctionType.Exp

    nc = bacc.Bacc("TRN2", target_bir_lowering=False, debug=False,
                   num_devices=NCORES)

    # host-packed layouts: each SBUF tile is one contiguous DRAM block so a
    # single dma_start with 64KB descriptors loads it (sequencer-cheap).
    xT_d = nc.dram_tensor("xTt", [B * 4 * 4 * 128, 2048], bf,
                          kind="ExternalInput")
    wq_d = nc.dram_tensor("wqt", [128, KT * HPC * D], bf,
                          kind="ExternalInput")
    wk_d = nc.dram_tensor("wkt", [128, KT * HPC * D], bf,
                          kind="ExternalInput")
    wv_d = nc.dram_tensor("wvt", [128, KT * HPC * D], bf,
                          kind="ExternalInput")
    cos_d = nc.dram_tensor("ccT", [D, T], bf, kind="ExternalInput")
    sin_d = nc.dram_tensor("ssT", [D, T], bf, kind="ExternalInput")
    mask_d = nc.dram_tensor("mask0", [128, 128], bf, kind="ExternalInput")
    wp_d = nc.dram_tensor("wpt", [4 * 128, KT * 512], bf,
                          kind="ExternalInput")
    # out rows: 256*q + 128*bs + p  <->  (b=core//2, t=512q+256(core%2)+...)
    out_d = nc.dram_tensor("outBT", [BTPC, C], bf, kind="ExternalOutput")

    with tile.TileContext(nc) as tc:
        from contextlib import ExitStack as _ES
        with _ES() as dstack:
            dpools = [dstack.enter_context(
                tc.tile_pool(name=f"dram{i}", bufs=1, space="DRAM"))
                for i in range(8)]
            a2a_in = [dpools[q].tile([2048, 256], bf, name=f"a2a_in{q}",
                                     tag=f"a2a_in{q}")
                      for q in range(4)]
            a2a_out = [dpools[4 + q].tile([2048, 256], bf,
                                          name=f"a2a_out{q}",
                                          tag=f"a2a_out{q}")
                       for q in range(4)]

            with ExitStack() as ab:
                # pools that live across both phases
                const = ab.enter_context(tc.tile_pool(name="const", bufs=1))
                rot_pool = ab.enter_context(
                    tc.tile_pool(name="rot", bufs=2 * B * HPC))
                v_pool = ab.enter_context(
                    tc.tile_pool(name="v", bufs=B * KT // 2))
                exp_pool = ab.enter_context(tc.tile_pool(name="expp",
                                                         bufs=4))
                acc_pool = ab.enter_context(tc.tile_pool(name="acc",
                                                         bufs=2))
                norm_pool = ab.enter_context(tc.tile_pool(name="norm",
                                                          bufs=2))

                ones_sb = const.tile([128, 128], bf, name="ones_sb")
                nc.vector.memset(ones_sb[:], 1.0)
                mask0_sb = const.tile([128, 128], bf, name="mask0_sb")
                warm_sb = const.tile([128, 128], bf, name="warm_sb")

                rot = {}
                vt = {}

                # ============ Phase 1: QKV + RoPE, all batches ============
                with ExitStack() as p1:
                    wpool = p1.enter_context(
                        tc.tile_pool(name="w", bufs=1))
                    xt_pool = p1.enter_context(
                        tc.tile_pool(name="xt", bufs=8))
                    qkraw_pool = p1.enter_context(
                        tc.tile_pool(name="qkraw", bufs=4))
                    rtmp_pool = p1.enter_context(
                        tc.tile_pool(name="rtmp", bufs=2))
                    c1 = p1.enter_context(tc.tile_pool(name="c1", bufs=1))
                    ps1 = p1.enter_context(
                        tc.tile_pool(name="ps1", bufs=1, space="PSUM"))

                    # one wide tile per weight tensor: free dim = kk
                    # blocks; DMA'd in 4 chunks so accumulation can start
                    # as soon as the first chunk + x tiles arrive
                    wqkv_sb = {}
                    CW = 4 * HPC * D
                    for (d_, tag) in ((wq_d, "wq"), (wk_d, "wk"),
                                      (wv_d, "wv")):
                        w_ = wpool.tile([128, KT * HPC * D], bf,
                                        name=f"{tag}_sb", tag=tag)
                        wqkv_sb[tag] = w_
                    wload = []
                    for ch in range(4):
                        for (d_, tag) in ((wq_d, "wq"), (wk_d, "wk"),
                                          (wv_d, "wv")):
                            wload.append((wqkv_sb[tag], d_, ch))

                    def load_w(items, eng=None):
                        # weight loads ride the scalar HWDGE queue so they
                        # issue in parallel with the x loads on sync
                        for (w_, d_, ch) in items:
                            (eng or nc.scalar).dma_start(
                                w_[:, CW * ch:CW * ch + CW],
                                d_.ap()[:, CW * ch:CW * ch + CW])

                    def wslice(tag, kk, l):
                        c0 = HPC * D * kk + 128 * l
                        return wqkv_sb[tag][:, c0:c0 + 128]

                    # xt tiles: [128, 2048] covering 4 kk blocks (b,tt4,kkg)
                    xt = {}

                    def load_xt1(b, tt4, kkg):
                        t_ = xt_pool.tile(
                            [128, 2048], bf,
                            name=f"xt_{b}_{tt4}_{kkg}", tag="xt")
                        r0 = 128 * (16 * b + 4 * tt4 + kkg)
                        nc.sync.dma_start(
                            t_[:], xT_d.ap()[r0:r0 + 128, :])
                        xt[(b, tt4, kkg)] = t_

                    def load_xt(b, tt4s=range(4)):
                        for tt4 in tt4s:
                            for kkg in range(4):
                                load_xt1(b, tt4, kkg)

                    def xslice(b, kk, tt4):
                        return xt[(b, tt4, kk // 4)][
                            :, 512 * (kk % 4):512 * (kk % 4) + 512]

                    def xwslice(b, kk, tt4, s):
                        c0 = 512 * (kk % 4) + 128 * s
                        return xt[(b, tt4, kk // 4)][:, c0:c0 + 128]

                    # three parallel DMA queues at startup: wq on scalar,
                    # x on sync, wk/wv on gpsimd -- so wk c0 lands before
                    # the first k-chain starts (~7us in) instead of
                    # queueing behind all of wq on one queue
                    load_w([x for x in wload if x[1] is wq_d])
                    load_xt(0, [0])
                    load_w([x for x in wload if x[1] is not wq_d],
                           eng=nc.gpsimd)
                    load_xt(0, [1, 2, 3])

                    cos_sb = c1.tile([D, T], bf, name="cos_sb")
                    nc.scalar.dma_start(cos_sb[:], cos_d.ap())
                    sin_sb = c1.tile([D, T], bf, name="sin_sb")
                    nc.scalar.dma_start(sin_sb[:], sin_d.ap())
                    nc.scalar.dma_start(mask0_sb[:], mask_d.ap())

                    # warm-up matmuls: the PE would otherwise sit idle for
                    # ~6us waiting on the first DMAs and then run its first
                    # ~3.4us of real matmuls at the cold 1.2 GHz HAM clock.
                    # Spinning on the ones tile gets HAM to K=8/8 for free.
                    wps = ps1.tile([128, 512], f32, name="warmps",
                                   tag="psqkv", bufs=8)
                    for wi in range(48):
                        nc.tensor.matmul(wps[:, 0:128], ones_sb[:],
                                         ones_sb[:], start=True, stop=True)
                    nc.vector.tensor_copy(warm_sb[:], wps[:, 0:128])

                    for b in range(B):
                        if b + 1 < B:
                            load_xt(b + 1)
                        qraw, kraw = {}, {}
                        for l in range(HPC):
                            qraw[l] = qkraw_pool.tile(
                                [128, T], bf, name=f"qraw_{b}_{l}",
                                tag="qkraw")
                            kraw[l] = qkraw_pool.tile(
                                [128, T], bf, name=f"kraw_{b}_{l}",
                                tag="qkraw")

                        for tt4 in range(4):
                            for l in range(HPC):
                                for (wtag, raw, nm) in (("wq", qraw[l], "q"),
                                                        ("wk", kraw[l],
                                                         "k")):
                                    ps = ps1.tile(
                                        [128, 512], f32,
                                        name=f"{nm}ps_{b}_{l}_{tt4}",
                                        tag="psqkv", bufs=8)
                                    for kk in range(KT):
                                        nc.tensor.matmul(
                                            ps[:],
                                            wslice(wtag, kk, l),
                                            xslice(b, kk, tt4),
                                            start=(kk == 0),
                                            stop=(kk == KT - 1))
                                    nc.scalar.copy(
                                        raw[:, 512 * tt4:512 * tt4 + 512],
                                        ps[:])
                            # v directly in [t, D] layout: x-tile slice is
                            # the stationary operand, Wv^T slices stream.
                            # Two 16-MM N=256 chains share one PSUM bank.
                            wv_sb = wqkv_sb["wv"]
                            for half in range(2):
                                vps = ps1.tile([128, 512], f32,
                                               name=f"vps_{b}_{tt4}_{half}",
                                               tag="psqkv", bufs=8)
                                for sub in range(2):
                                    s = 2 * half + sub
                                    for kk in range(KT):
                                        nc.tensor.matmul(
                                            vps[:, 256 * sub:256 * sub + 256],
                                            xwslice(b, kk, tt4, s),
                                            wv_sb[:, 256 * kk:256 * kk + 256],
                                            start=(kk == 0),
                                            stop=(kk == KT - 1))
                                v_ = v_pool.tile([128, 512], bf,
                                                 name=f"v_{b}_{tt4}_{half}",
                                                 tag="v")
                                nc.scalar.copy(v_[:], vps[:])
                                for sub in range(2):
                                    tk = 4 * tt4 + 2 * half + sub
                                    vt[(b, tk)] = (v_, sub)

                        # ---- RoPE ----
                        for l in range(HPC):
                            for (raw, which) in ((qraw[l], "q"),
                                                 (kraw[l], "k")):
                                sw = rtmp_pool.tile(
                                    [128, T], bf,
                                    name=f"sw_{which}_{b}_{l}", tag="sw")
                                # gpsimd queue: decoupled from the x-tile
                                # prefetch stream on sync, which can be
                                # ring-blocked for tens of us
                                nc.gpsimd.dma_start(sw[0:64, :],
                                                    raw[64:128, :])
                                nc.gpsimd.dma_start(sw[64:128, :],
                                                    raw[0:64, :])
                                r_ = rot_pool.tile(
                                    [128, T], bf,
                                    name=f"rot_{which}_{b}_{l}", tag="rot")
                                nc.vector.tensor_tensor(
                                    r_[:], raw[:], cos_sb[:], op=mult)
                                nc.vector.tensor_tensor(
                                    sw[:], sw[:], sin_sb[:], op=mult)
                                nc.vector.tensor_tensor(
                                    r_[:], r_[:], sw[:], op=add)
                                rot[(b, l, which)] = r_

                # ============ Phase 2: attention quarters + out-proj ======
                with ExitStack() as p2:
                    wp_pool = p2.enter_context(
                        tc.tile_pool(name="wp", bufs=4))
                    y2_pool = p2.enter_context(
                        tc.tile_pool(name="y2", bufs=30))
                    ob_pool = p2.enter_context(
                        tc.tile_pool(name="ob", bufs=2))
                    ps2 = p2.enter_context(
                        tc.tile_pool(name="ps2", bufs=1, space="PSUM"))

                    # wp: one wide tile per cc chunk, free dim = kk blocks.
                    # Loaded on the gpsimd queue AFTER the A2A0 trigger so
                    # the 8.4MB doesn't contend with the first collective.
                    wp_wide = []

                    def load_wp():
                        for cc in range(4):
                            w_ = wp_pool.tile([128, KT * 512], bf,
                                              name=f"wp_{cc}", tag="wp")
                            nc.gpsimd.dma_start(
                                w_[:], wp_d.ap()[128 * cc:128 * cc + 128, :])
                            wp_wide.append(w_)

                    def wpslice(kk, cc):
                        return wp_wide[cc][:, 512 * kk:512 * kk + 512]

                    y2_tiles = {}

                    def load_y2(qq):
                        # sync queue, emitted only once its A2A is known
                        # done so it never head-of-line-blocks yn stores
                        tiles = []
                        for kk in range(KT):
                            y_ = y2_pool.tile([128, 256], bf,
                                              name=f"y2_{qq}_{kk}", tag="y2")
                            nc.sync.dma_start(
                                y_[:],
                                a2a_out[qq][128 * kk:128 * kk + 128, :])
                            tiles.append(y_)
                        y2_tiles[qq] = tiles

                    def emit_attn_pair(b, l, jj):
                        qrot = rot[(b, l, "q")]
                        krot = rot[(b, l, "k")]
                        q0 = 512 * jj
                        yps = ps2.tile([128, 512], f32,
                                       name=f"yps_{b}_{l}_{jj}", tag="psy",
                                       bufs=2)
                        acc = acc_pool.tile([128, 512], bf,
                                            name=f"acc_{b}_{l}_{jj}",
                                            tag="acc", bufs=3)
                        acc2 = None
                        groups = []
                        for g in range(2 * jj):
                            groups.append([(2 * g, 0, 512, 0),
                                           (2 * g + 1, 512, 512, 0)])
                        groups.append([(4 * jj + 0, 0, 512, 0),
                                       (4 * jj + 1, 512, 384, 128)])
                        groups.append([(4 * jj + 2, 0, 256, 256),
                                       (4 * jj + 3, 256, 128, 384)])
                        ng = len(groups)
                        first = True
                        for gi, parts in enumerate(groups):
                            tot_w = parts[-1][1] + parts[-1][2]
                            scps = ps2.tile([128, 1024], f32,
                                            name=f"sc_{b}_{l}_{jj}_{gi}",
                                            tag="pssc", bufs=2)
                            for (tk, co, w, qo) in parts:
                                nc.tensor.matmul(
                                    scps[:, co:co + w],
                                    krot[:, 128 * tk:128 * tk + 128],
                                    qrot[:, q0 + qo:q0 + 512],
                                    start=True, stop=True)
                            ex = exp_pool.tile(
                                [128, 1024], bf,
                                name=f"ex_{b}_{l}_{jj}_{gi}", tag="ex")
                            nc.scalar.activation(
                                ex[:, 0:tot_w], scps[:, 0:tot_w], Exp,
                                scale=SCALE)
                            if gi >= 2 * jj:  # diagonal group
                                for (tk, co, w, qo) in parts:
                                    nc.vector.tensor_tensor(
                                        ex[:, co:co + 128],
                                        ex[:, co:co + 128],
                                        mask0_sb[:], op=mult)
                            for (tk, co, w, qo) in parts:
                                if first:
                                    nc.vector.tensor_copy(
                                        acc[:], ex[:, co:co + w])
                                    first = False
                                else:
                                    nc.vector.tensor_tensor(
                                        acc[:, qo:512], acc[:, qo:512],
                                        ex[:, co:co + w], op=add)
                            for pi_, (tk, co, w, qo) in enumerate(parts):
                                v_, sub = vt[(b, tk)]
                                vc0 = 256 * sub + 128 * l
                                nc.tensor.matmul(
                                    yps[:, qo:512],
                                    v_[:, vc0:vc0 + 128],
                                    ex[:, co:co + w],
                                    start=(gi == 0 and pi_ == 0),
                                    stop=(gi == ng - 1 and
                                          pi_ == len(parts) - 1),
                                    skip_group_check=True)
                        def finalize(b=b, l=l, jj=jj, acc=acc, acc2=acc2,
                                     yps=yps):
                            if acc2 is not None:
                                nc.vector.tensor_tensor(acc[:], acc[:],
                                                        acc2[:], op=add)
                            sums = ps2.tile([128, 512], f32,
                                            name=f"sums_{b}_{l}_{jj}",
                                            tag="psop", bufs=2)
                            nc.tensor.matmul(sums[:], ones_sb[:], acc[:],
                                             start=True, stop=True)
                            rec = norm_pool.tile([128, 512], f32,
                                                 name=f"rec_{b}_{l}_{jj}",
                                                 tag="rec", bufs=2)
                            nc.vector.reciprocal_approx_fast(rec[:],
                                                             sums[:])
                            yn = norm_pool.tile([128, 512], bf,
                                                name=f"yn_{b}_{l}_{jj}",
                                                tag="yn", bufs=6)
                            nc.vector.tensor_tensor(yn[:], yps[:], rec[:],
                                                    op=mult)
                            for h in range(2):
                                r0 = 256 * (2 * b + h) + 128 * l
                                nc.sync.dma_start(
                                    a2a_in[jj][r0:r0 + 128, :],
                                    yn[:, 256 * h:256 * h + 256])
                        return finalize

                    def emit_outproj_group(qq, bs, cc, split_store=False):
                        ps = ps2.tile([128, 512], f32,
                                      name=f"ops_{qq}_{bs}_{cc}", tag="psop",
                                      bufs=2)
                        y2t = y2_tiles[qq]
                        for kk in range(KT):
                            nc.tensor.matmul(
                                ps[:],
                                y2t[kk][:, 128 * bs:128 * bs + 128],
                                wpslice(kk, cc),
                                start=(kk == 0), stop=(kk == KT - 1))
                        ob = ob_pool.tile([128, 512], bf,
                                          name=f"ob_{qq}_{bs}_{cc}",
                                          tag="ob")
                        r0 = 256 * qq + 128 * bs
                        # the very last group's eviction+store is fully
                        # exposed at the kernel tail: split it so the
                        # second half's copy overlaps the first's store
                        nparts = 2 if split_store else 1
                        for h in range(nparts):
                            c0 = 512 * h // nparts
                            c1 = 512 * (h + 1) // nparts
                            nc.vector.tensor_copy(ob[:, c0:c1],
                                                  ps[:, c0:c1])
                            nc.sync.dma_start(
                                out_d.ap()[r0:r0 + 128,
                                           512 * cc + c0:512 * cc + c1],
                                ob[:, c0:c1])

                    pairs = [(b, l) for b in range(B) for l in range(HPC)]
                    op_groups = [(bs, cc) for bs in range(2)
                                 for cc in range(4)]
                    # op(0) starts filling PE bubbles late in jj=2 (A2A0 is
                    # done by then); the rest of op(0) + op(1) interleave
                    # into jj=3 over pairs 0-6 (pair 7 gets none so its
                    # finalize -- and thus the A2A3 trigger -- is not pushed
                    # back). op(2) is NOT interleaved: it runs as one block
                    # after the A2A3 trigger, covering the A2A3 latency.
                    sched = {}
                    flat = ([(0, g) for g in range(8)] +
                            [(1, g) for g in range(4)])
                    slot = 0
                    for pi_ in range(7):
                        take = 2 if pi_ < 5 else 1
                        for _ in range(take):
                            if slot < len(flat):
                                sched.setdefault((3, pi_), []).append(
                                    flat[slot])
                                slot += 1

                    def emit_coll(q):
                        nc.gpsimd.collective_compute(
                            "AllToAll",
                            mybir.AluOpType.bypass,
                            replica_groups=[list(range(NCORES))],
                            ins=[a2a_in[q].opt()],
                            outs=[a2a_out[q].opt()],
                        )

                    # collective emission is DELAYED past the point where
                    # its sync-queue wait (previous collective done) is
                    # already satisfied, so it never blocks yn writes.
                    pending = None
                    for jj in range(4):
                        for pi, (b, l) in enumerate(pairs):
                            fin = emit_attn_pair(b, l, jj)
                            if pending is not None:
                                pending()
                            pending = fin
                            if jj == 2 and pi == 3:
                                emit_coll(1)
                            if jj == 3 and pi == 0:
                                emit_coll(2)
                            for (qq, g) in sched.get((jj, pi), []):
                                if g == 0:
                                    load_y2(qq)
                                bs, cc = op_groups[g]
                                emit_outproj_group(qq, bs, cc)
                        pending()
                        pending = None
                        if jj == 0:
                            emit_coll(0)
                            load_wp()

                    emit_coll(3)
                    # op(1) second half + all of op(2) (50us of PE work)
                    # run during the A2A3 wait -- covers even a slow A2A
                    for g in range(4, 8):
                        bs, cc = op_groups[g]
                        emit_outproj_group(1, bs, cc)
                    load_y2(2)
                    for (bs, cc) in op_groups:
                        emit_outproj_group(2, bs, cc)
                    load_y2(3)
                    for gi_, (bs, cc) in enumerate(op_groups):
                        emit_outproj_group(3, bs, cc,
                                           split_store=(gi_ == 7))

    nc.compile()
    return nc


def _prep_inputs(x, rope_freqs, W_attn, W_proj):
    x = np.asarray(x, np.float32)
    rope_freqs = np.asarray(rope_freqs, np.float32)
    W_attn = np.asarray(W_attn, np.float32)
    W_proj = np.asarray(W_proj, np.float32)

    xT = x.reshape(BT, C).T.astype(BF16)            # (C, BT)
    # pack per-[128,2048] tile (b, tt4, kkg): tile[p, 512*k4+j] =
    # xT[128*(4*kkg+k4)+p, 2048*b+512*tt4+j]
    xTt = np.ascontiguousarray(
        xT.reshape(4, 4, 128, B, 4, 512).transpose(3, 4, 0, 2, 1, 5)
    ).reshape(-1, 2048)
    perm = np.concatenate([np.arange(0, D, 2), np.arange(1, D, 2)])
    theta = np.outer(rope_freqs.astype(np.float64), np.arange(T))
    cos_, sin_ = np.cos(theta), np.sin(theta)
    ccT = np.concatenate([cos_, cos_], axis=0).astype(BF16)   # (128, T)
    ssT = np.concatenate([-sin_, sin_], axis=0).astype(BF16)  # (128, T)
    mask0 = (np.arange(128)[None, :] >= np.arange(128)[:, None])
    mask0 = mask0.astype(np.float32).astype(BF16)
    wpT = W_proj.T.astype(BF16)                     # (C, C)
    # wpt[128*cc+p, 512*kk+j] = wpT[128*kk+p, 512*cc+j]
    wpt = np.ascontiguousarray(
        wpT.reshape(KT, 128, 4, 512).transpose(2, 1, 0, 3)
    ).reshape(4 * 128, KT * 512)

    def packw(wT):  # (C, 256) -> [128, KT*256]: out[p, 256*kk+m]=wT[128kk+p,m]
        return np.ascontiguousarray(
            wT.reshape(KT, 128, HPC * D).transpose(1, 0, 2)
        ).reshape(128, KT * HPC * D)

    in_maps = []
    for r in range(NCORES):
        wq_rows, wk_rows, wv_rows = [], [], []
        for l in range(HPC):
            h = HPC * r + l
            wq_rows.append(W_attn[D * h:D * h + D][perm])
            wk_rows.append(W_attn[C + D * h:C + D * h + D][perm])
            wv_rows.append(W_attn[2 * C + D * h:2 * C + D * h + D])
        in_maps.append({
            "xTt": xTt,
            "wqt": packw(np.concatenate(wq_rows, 0).T.astype(BF16)),
            "wkt": packw(np.concatenate(wk_rows, 0).T.astype(BF16)),
            "wvt": packw(np.concatenate(wv_rows, 0).T.astype(BF16)),
            "ccT": ccT,
            "ssT": ssT,
            "mask0": mask0,
            "wpt": wpt,
        })
    return in_maps


def _ensure_trace_support():
    """Register the axon NTFF profiling hook if the image's antenv lacks it,
    and stub out the artifact upload (no bucket access in-container)."""
    import types
    import sys as _sys
    import antenv

    if "antenv.axon_hooks" not in _sys.modules:
        try:
            import antenv.axon_hooks  # noqa: F401
        except ImportError:
            mod = types.ModuleType("antenv.axon_hooks")
            _holder = {}
            mod.set_axon_ntff_profile_hook = (
                lambda h: _holder.__setitem__("h", h))
            mod.get_axon_ntff_profile_hook = lambda: _holder.get("h")
            _sys.modules["antenv.axon_hooks"] = mod
            antenv.axon_hooks = mod
    import antenv.axon_hooks as ah

    if ah.get_axon_ntff_profile_hook() is None:
        try:
            from trn_agent_boot.trn_boot import _ntff_profile_via_ctypes
            hook = _ntff_profile_via_ctypes("/opt/axon/libaxon_pjrt.so")
            if hook is not None:
                ah.set_axon_ntff_profile_hook(hook)
        except Exception as e:  # profiling stays off; run still works
            print(f"ntff hook registration failed: {e}", file=sys.stderr)
    from concourse import bass_utils as bu
    bu.upload_artifacts = lambda tmpdir: f"local://{tmpdir}"


def kernel(x, rope_freqs, W_attn, W_proj):
    global LAST_EXEC_NS
    from concourse import bass_utils

    if "nc" not in _CACHE:
        _CACHE["nc"] = _build_nc()
    nc = _CACHE["nc"]

    in_maps = _prep_inputs(x, rope_freqs, W_attn, W_proj)
    trace = os.environ.get("KERNEL_TRACE", "0") == "1"
    tmpdir = None
    if trace:
        _ensure_trace_support()
        tmpdir = os.environ.get("KERNEL_TRACE_DIR") or None
    tcores = os.environ.get("KERNEL_TRACE_CORES")
    tcores = [int(x) for x in tcores.split(",")] if tcores else None
    res = bass_utils.run_bass_kernel_spmd(
        nc, in_maps, core_ids=list(range(NCORES)), trace=trace,
        tmpdir=tmpdir, trace_cores=tcores)
    LAST_EXEC_NS = res.exec_time_ns

    # core c rows: 256*q + i  <->  (b=c//2, t=512*q+256*(c%2)+i)
    out = np.empty((B, T, C), np.float32)
    for c in range(NCORES):
        oc = np.asarray(res.results[c]["outBT"], np.float32)
        bb = c // 2
        for q in range(4):
            t0 = 512 * q + 256 * (c % 2)
            out[bb, t0:t0 + 256, :] = oc[256 * q:256 * q + 256, :]
    return out

